# revision 36
# baseline (speedup 1.0000x reference)
"""HLMPNN (hierarchical layered MPNN) Bass kernel for 8 TRN2 NeuronCores.

Strategy (graph/data parallel, per sharding hint):
  - Nodes partitioned row-wise across 8 cores (6250 each, padded to 6272).
  - Edge MLP decomposed: msg = relu(z[src]@W1+b1)@W2+b2 with per-node
    Q = relu(z@W1+b1), so segment_mean(msg) = (segsum(Q[src])/c)@W2 + b2.
  - Per layer: compute own Q shard in two row-chunks (A=3200, B=3072 rows),
    AllGather each chunk separately (chunk A's collective overlaps chunk B's
    compute; chunk B's collective overlaps pass-A gather/scatter).
  - Edges sorted by (src-chunk, dst-group); per chunk, gathers are fused into
    32-block (4096-index) dma_gather windows; scatter-add via one-hot matmuls
    (bf16) accumulating in PSUM, pass-A partials parked in SBUF.
  - Node MLP + LayerNorm per group; beta-weighted sum accumulated into the
    output DRAM buffer. Matmul operands bf16, accumulation/LN fp32.
"""
import math
import numpy as np
import ml_dtypes

import concourse.bass as bass
import concourse.bass2jax as _b2j
import concourse.mybir as mybir

_orig_hook = _b2j.neuronx_cc_hook
def _dbg_hook(*a, **k):
    try:
        return _orig_hook(*a, **k)
    except BaseException:
        import traceback
        traceback.print_exc()
        raise
_b2j.neuronx_cc_hook = _dbg_hook
import concourse.tile as tile
from concourse import bacc
from concourse.bass_utils import run_bass_kernel_spmd
from concourse.masks import make_identity

F32 = mybir.dt.float32
BF16 = mybir.dt.bfloat16
I16 = mybir.dt.int16
AF = mybir.ActivationFunctionType
OP = mybir.AluOpType
NPBF = ml_dtypes.bfloat16

CORES = 8
N = 50000
IN_CH = 128
HID = 256
MSG = 128
L = 10
EPS = 1e-5
NPC = N // CORES            # 6250
G = (NPC + 127) // 128      # 49
NPAD = G * 128              # 6272
GA = 25                     # groups in row-chunk A
CA = GA * 128               # 3200 rows
CB = NPAD - CA              # 3072 rows
TA = CORES * CA             # 25600 (chunk-A gather table rows)
TB = CORES * CB             # 24576
PADDST = 200.0
WIN = 32                    # blocks per fused dma_gather window


def _preprocess(edge_index):
    src = np.asarray(edge_index[0], np.int64)
    dst = np.asarray(edge_index[1], np.int64)
    loops = np.arange(N, dtype=np.int64)
    src = np.concatenate([src, loops])
    dst = np.concatenate([dst, loops])

    score = src // NPC
    sloc = src % NPC
    tchunk = (sloc >= CA).astype(np.int64)
    idxval = np.where(tchunk == 0, score * CA + sloc, score * CB + (sloc - CA))

    owner = dst // NPC
    dloc = dst % NPC
    grp = dloc // 128
    nloc = dloc % 128

    counts = np.zeros((CORES, NPAD), np.float32)
    np.add.at(counts, (owner, dloc), 1.0)
    cinv = np.zeros_like(counts)
    nz = counts > 0
    cinv[nz] = 1.0 / counts[nz]

    order = np.lexsort((idxval, grp, tchunk, owner))
    so, st, sg = owner[order], tchunk[order], grp[order]
    si, sn = idxval[order], nloc[order]
    key = (so * 2 + st) * G + sg
    bounds = np.searchsorted(key, np.arange(CORES * 2 * G + 1))

    B = np.zeros((2, G), np.int64)  # unified block counts [chunk, group]
    for r in range(CORES):
        for t in range(2):
            for g in range(G):
                k = (r * 2 + t) * G + g
                B[t, g] = max(B[t, g], -(-(bounds[k + 1] - bounds[k]) // 128))
    seg_off = np.zeros((2, G), np.int64)
    off = 0
    for t in range(2):
        for g in range(G):
            seg_off[t, g] = off
            off += int(B[t, g]) * 128
    totslots = off
    totb = totslots // 128

    idx16 = np.zeros((CORES, totslots), np.int16)
    dstv = np.full((CORES, totslots), PADDST, np.float32)
    for r in range(CORES):
        for t in range(2):
            for g in range(G):
                k = (r * 2 + t) * G + g
                lo, hi = bounds[k], bounds[k + 1]
                o = seg_off[t, g]
                idx16[r, o:o + hi - lo] = si[lo:hi].astype(np.int16)
                dstv[r, o:o + hi - lo] = sn[lo:hi].astype(np.float32)

    # packed gather-index layout: slot i -> [i%16, i//16], replicated x8 partitions
    idx_pack = np.tile(
        idx16.reshape(CORES, totslots // 16, 16).transpose(0, 2, 1), (1, 8, 1)
    )  # [CORES, 128, totslots//16]
    dstv_cols = dstv.reshape(CORES, totb, 128).transpose(0, 2, 1)  # [CORES,128,totb]
    return B, seg_off, idx_pack, dstv_cols, cinv, totb


def _make_windows(B):
    """Per chunk: fused gather windows [(global_block0, nblocks)]."""
    wins = []
    nb0 = int(B[0].sum())
    for t in range(2):
        lo = 0 if t == 0 else nb0
        hi = lo + int(B[t].sum())
        w = []
        b = lo
        while b < hi:
            nb = min(WIN, hi - b)
            w.append((b, nb))
            b += nb
        wins.append(w)
    return wins


NQ = 4  # SWDGE DMA queues for gather round-robin


def _build(B, seg_off, totb, betas, nlayers=L):
    nc = bacc.Bacc(None, target_bir_lowering=False, debug=False,
                   num_swdge_queues=NQ)
    wins = _make_windows(B)

    xT = nc.dram_tensor("xT", [128, NPAD], F32, kind="ExternalInput")
    win_d = nc.dram_tensor("win", [128, HID], F32, kind="ExternalInput")
    binrow_d = nc.dram_tensor("binrow", [1, HID], F32, kind="ExternalInput")
    w1_d = nc.dram_tensor("w1t", [nlayers, 128, 2, 128], BF16, kind="ExternalInput")
    w2_d = nc.dram_tensor("w2t", [nlayers, 128, 2, 128], BF16, kind="ExternalInput")
    u1_d = nc.dram_tensor("u1t", [nlayers, 128, 4, 128], BF16, kind="ExternalInput")
    u2_d = nc.dram_tensor("u2t", [nlayers, 128, 2, HID], BF16, kind="ExternalInput")
    cols_d = nc.dram_tensor("cols", [nlayers, 128, 8], F32, kind="ExternalInput")
    rowsb_d = nc.dram_tensor("rowsb", [nlayers, 4 * HID], BF16, kind="ExternalInput")
    idx_d = nc.dram_tensor("idxp", [128, totb * 8], I16, kind="ExternalInput")
    dstv_d = nc.dram_tensor("dstv", [128, totb], BF16, kind="ExternalInput")
    cinv_d = nc.dram_tensor("cinv", [128, NPAD], BF16, kind="ExternalInput")
    # output accumulator, feature-major: out[p, f*NPAD + n] = feat (f*128+p) of node n
    out_d = nc.dram_tensor("out", [128, 2 * NPAD], F32, kind="ExternalOutput")

    ASZ = int(B.max())  # max blocks per (chunk, group) segment

    with tile.TileContext(nc) as tc:
        with (
            tc.tile_pool(name="persist", bufs=1) as pp,
            tc.tile_pool(name="wpool", bufs=2) as wp,
            tc.tile_pool(name="stream", bufs=2) as sp,
            tc.tile_pool(name="gath", bufs=3) as gp,
            tc.tile_pool(name="psum", bufs=1, space="PSUM") as psp,
            tc.tile_pool(name="dram", bufs=2, space="DRAM") as dp,
            tc.tile_pool(name="dramp", bufs=1, space="DRAM") as dq,
        ):
            # ---- persistent state ----
            zT = pp.tile([128, 2 * NPAD], BF16)        # state, feature-major
            oacc = pp.tile([128, 2 * NPAD], BF16)      # output accum, feature-major
            S_sb = pp.tile([128, G * 128], F32)        # pass-A scatter partials
            cinv_sb = pp.tile([128, NPAD], BF16)
            dstv_sb = pp.tile([128, totb], BF16)
            idx_sb = pp.tile([128, totb * 8], I16)
            iota_bf = pp.tile([128, 128], BF16)
            ident = pp.tile([128, 128], F32)
            ident_bf = pp.tile([128, 128], BF16)
            ones_r = pp.tile([1, 128], F32)
            ones_bf = pp.tile([1, 128], BF16)
            eps_sb = pp.tile([128, 1], F32)
            win_sb = pp.tile([128, HID], F32)
            binrow_sb = pp.tile([1, HID], F32)

            nc.sync.dma_start(out=cinv_sb[:], in_=cinv_d[:])
            nc.sync.dma_start(out=dstv_sb[:], in_=dstv_d[:])
            nc.sync.dma_start(out=idx_sb[:], in_=idx_d[:])
            nc.sync.dma_start(out=win_sb[:], in_=win_d[:])
            nc.sync.dma_start(out=binrow_sb[:], in_=binrow_d[:])

            iota_i = sp.tile([128, 128], mybir.dt.int32, tag="ioi")
            nc.gpsimd.iota(iota_i[:], pattern=[[1, 128]], base=0, channel_multiplier=0)
            nc.vector.tensor_copy(out=iota_bf[:], in_=iota_i[:])
            make_identity(nc, ident[:])
            make_identity(nc, ident_bf[:])
            nc.vector.memset(ones_r[:], 1.0)
            nc.vector.memset(ones_bf[:], 1.0)
            nc.vector.memset(eps_sb[:], EPS)

            def fcols(f, g):
                return slice(f * NPAD + g * 128, f * NPAD + (g + 1) * 128)

            def outv(g):
                return out_d[:].rearrange("p (f n) -> p f n", f=2)[
                    :, :, g * 128:(g + 1) * 128
                ]

            # zT 2-half view for a group: [128, 2, 128]
            def z_pair(g):
                return zT[:].rearrange("p (h n) -> p h n", h=2)[
                    :, :, g * 128:(g + 1) * 128
                ]

            # ---- z0 ----
            for g in range(G):
                xg = sp.tile([128, 128], F32, tag="xg")
                nc.sync.dma_start(out=xg[:], in_=xT[:, g * 128:(g + 1) * 128])
                zq = psp.tile([128, HID], F32, tag="mp", space="PSUM")
                nc.tensor.matmul(zq[:], lhsT=xg[:], rhs=win_sb[:], start=True, stop=False)
                nc.tensor.matmul(zq[:], lhsT=ones_r[:], rhs=binrow_sb[:], start=False, stop=True)
                z0 = sp.tile([128, HID], F32, tag="z0")
                nc.vector.tensor_copy(out=z0[:], in_=zq[:])
                ztp = psp.tile([128, HID], F32, tag="ztp", space="PSUM")
                for f in range(2):
                    nc.tensor.transpose(
                        out=ztp[:, f * 128:(f + 1) * 128],
                        in_=z0[:, f * 128:(f + 1) * 128],
                        identity=ident[:],
                    )
                    nc.vector.tensor_copy(out=zT[:, fcols(f, g)],
                                          in_=ztp[:, f * 128:(f + 1) * 128])
                for f in range(2):
                    nc.scalar.activation(oacc[:, fcols(f, g)],
                                         ztp[:, f * 128:(f + 1) * 128],
                                         AF.Copy, scale=float(betas[0]))

            # ---- layers (software-pipelined: Q-chunk-A + AllGather-A of
            # layer l+1 are emitted inside layer l's pass B so the collective
            # hides behind scatter/MLP compute) ----
            def load_weights(l):
                w1t_sb = wp.tile([128, 2, 128], BF16, tag="w1")
                w2t_sb = wp.tile([128, 2, 128], BF16, tag="w2")
                u1t_sb = wp.tile([128, 4, 128], BF16, tag="u1")
                u2t_sb = wp.tile([128, 2, HID], BF16, tag="u2")
                rb_sb = wp.tile([1, 4 * HID], BF16, tag="rowsb")
                cl_sb = wp.tile([128, 8], F32, tag="cols")
                wt = {"w1": w1t_sb, "w2": w2t_sb, "u1": u1t_sb,
                      "u2": u2t_sb, "rowsb": rb_sb, "cols": cl_sb}
                nc.sync.dma_start(out=wt["w1"][:], in_=w1_d[l])
                nc.sync.dma_start(out=wt["w2"][:], in_=w2_d[l])
                nc.sync.dma_start(out=wt["u1"][:], in_=u1_d[l])
                nc.sync.dma_start(out=wt["u2"][:], in_=u2_d[l])
                nc.sync.dma_start(out=wt["rowsb"][:], in_=rowsb_d[l:l + 1, :])
                nc.sync.dma_start(out=wt["cols"][:], in_=cols_d[l])
                return wt

            def q_group(g, qown, base, wt):
                qp = psp.tile([128, MSG], F32, tag="qp", space="PSUM", bufs=2)
                nc.tensor.matmul(qp[:], lhsT=zT[:, fcols(0, g)],
                                 rhs=wt["w1"][:, 0, :], start=True, stop=False)
                nc.tensor.matmul(qp[:], lhsT=zT[:, fcols(1, g)],
                                 rhs=wt["w1"][:, 1, :], start=False, stop=False)
                nc.tensor.matmul(qp[:], lhsT=ones_bf[:],
                                 rhs=wt["rowsb"][:, 0:128],
                                 start=False, stop=True)
                q_sb = sp.tile([128, MSG], BF16, tag="q")
                nc.scalar.activation(q_sb[:], qp[:], AF.Relu)
                o = g * 128 - base
                nc.sync.dma_start(out=qown[o:o + 128, :], in_=q_sb[:])

            def emit_qa_groups(wt):
                qownA = dp.tile([CA, MSG], BF16, tag="qownA")
                qfullA = dp.tile([TA, MSG], BF16, tag="qfullA",
                                 addr_space="Shared")
                for g in range(GA):
                    q_group(g, qownA, 0, wt)
                return qownA, qfullA

            def emit_qa_collective(qownA, qfullA):
                nc.gpsimd.collective_compute(
                    "AllGather", OP.bypass,
                    replica_groups=[list(range(CORES))],
                    ins=[qownA[:].opt()], outs=[qfullA[:].opt()],
                )
                return qfullA

            def emit_qa(wt):
                qownA, qfullA = emit_qa_groups(wt)
                return emit_qa_collective(qownA, qfullA)

            wt_cur = load_weights(0)
            qfullA_cur = emit_qa(wt_cur)
            for l in range(nlayers):
                w1_sb, w2_sb = wt_cur["w1"], wt_cur["w2"]
                u1_sb, u2_sb = wt_cur["u1"], wt_cur["u2"]
                rowsb_sb, cols_sb = wt_cur["rowsb"], wt_cur["cols"]
                wt_next = load_weights(l + 1) if l + 1 < nlayers else None

                def rowb(i, lo=0, n=HID):
                    return rowsb_sb[:, i * HID + lo: i * HID + lo + n]

                # Q chunk B + its AllGather (chunk A was emitted during the
                # previous layer's pass B, or above for l=0)
                qownB = dp.tile([CB, MSG], BF16, tag="qownB")
                qfullB = dp.tile([TB, MSG], BF16, tag="qfullB", addr_space="Shared")
                for g in range(GA, G):
                    q_group(g, qownB, CA, wt_cur)
                nc.gpsimd.collective_compute(
                    "AllGather", OP.bypass,
                    replica_groups=[list(range(CORES))],
                    ins=[qownB[:].opt()], outs=[qfullB[:].opt()],
                )

                # ---- scatter passes ----
                def do_pass(t, qfull, finish, hooks=()):
                    hookmap = dict(hooks)
                    win_list = wins[t]
                    gat_by_win = []   # (b0, nb, tile)
                    wi = 0
                    for g in range(G):
                        nbseg = int(B[t, g])
                        s0 = int(seg_off[t, g]) // 128
                        fn = hookmap.pop(g, None)
                        if fn is not None:
                            fn()
                        while wi < len(win_list) and win_list[wi][0] < s0 + nbseg:
                            b0, nb = win_list[wi]
                            gat = gp.tile([128, WIN, 128], BF16, tag="gat", bufs=7)
                            nc.gpsimd.dma_gather(
                                out_ap=gat[:, :nb, :],
                                in_ap=qfull[:],
                                idxs_ap=idx_sb[:, b0 * 8:(b0 + nb) * 8],
                                num_idxs=nb * 128,
                                num_idxs_reg=nb * 128,
                                elem_size=MSG,
                                single_packet=False,
                                queue_num=wi % NQ,
                            )
                            gat_by_win.append((b0, nb, gat))
                            wi += 1
                        if nbseg == 0:
                            finish(g, None)
                            continue
                        a_sb = gp.tile([128, ASZ, 128], BF16, tag="a")
                        nc.vector.tensor_tensor(
                            out=a_sb[:, :nbseg, :],
                            in0=iota_bf[:, None, :].to_broadcast([128, nbseg, 128]),
                            in1=dstv_sb[:, s0:s0 + nbseg, None].to_broadcast(
                                [128, nbseg, 128]),
                            op=OP.is_equal,
                        )
                        sT = psp.tile([128, 128], F32, tag="sp", space="PSUM", bufs=2)
                        for j in range(nbseg):
                            bi = s0 + j
                            for (b0, nb, gat) in reversed(gat_by_win):
                                if b0 <= bi < b0 + nb:
                                    break
                            nc.tensor.matmul(
                                sT[:], lhsT=gat[:, bi - b0, :],
                                rhs=a_sb[:, j, :],
                                start=(j == 0), stop=(j == nbseg - 1),
                            )
                        finish(g, sT)

                def finish_a(g, sT):
                    gc = slice(g * 128, (g + 1) * 128)
                    if sT is None:
                        nc.vector.memset(S_sb[:, gc], 0.0)
                    else:
                        nc.vector.tensor_copy(out=S_sb[:, gc], in_=sT[:])

                def stage1(g, sT):
                    gc = slice(g * 128, (g + 1) * 128)
                    snorm = sp.tile([128, 128], BF16, tag="sn")
                    if sT is None:
                        nc.vector.tensor_tensor(
                            out=snorm[:], in0=S_sb[:, gc],
                            in1=cinv_sb[:, gc], op=OP.mult)
                    else:
                        ssum = sp.tile([128, 128], F32, tag="ssum")
                        nc.vector.tensor_tensor(
                            out=ssum[:], in0=S_sb[:, gc], in1=sT[:], op=OP.add)
                        nc.vector.tensor_tensor(
                            out=snorm[:], in0=ssum[:],
                            in1=cinv_sb[:, gc], op=OP.mult)

                    # m^T (two hid halves side by side) + b2, then h = z + m
                    mp = psp.tile([128, HID], F32, tag="mp", space="PSUM")
                    for m in range(2):
                        ms = slice(m * 128, (m + 1) * 128)
                        nc.tensor.matmul(mp[:, ms], lhsT=w2_sb[:, m, :], rhs=snorm[:],
                                         start=True, stop=False)
                        nc.tensor.matmul(mp[:, ms], lhsT=rowb(1, m * 128, 128),
                                         rhs=ones_bf[:], start=False, stop=True)
                    hT = sp.tile([128, HID], BF16, tag="h")
                    nc.vector.tensor_tensor(
                        out=hT[:].rearrange("p (h n) -> p h n", h=2),
                        in0=mp[:].rearrange("p (h n) -> p h n", h=2),
                        in1=z_pair(g),
                        op=OP.add,
                    )

                    # r = relu(U1.T @ h + c1)
                    rp = psp.tile([128, HID], F32, tag="rp", space="PSUM")
                    for m in range(2):
                        ms = slice(m * 128, (m + 1) * 128)
                        nc.tensor.matmul(rp[:, ms], lhsT=u1_sb[:, 0 * 2 + m, :],
                                         rhs=hT[:, 0:128], start=True, stop=False)
                        nc.tensor.matmul(rp[:, ms], lhsT=u1_sb[:, 1 * 2 + m, :],
                                         rhs=hT[:, 128:256], start=False, stop=False)
                        nc.tensor.matmul(rp[:, ms], lhsT=rowb(2, m * 128, 128),
                                         rhs=ones_bf[:], start=False, stop=True)
                    rT = sp.tile([128, HID], BF16, tag="rt")
                    nc.scalar.activation(rT[:], rp[:], AF.Relu)

                    # o = r @ U2 + c2 (node-major)
                    op_ = psp.tile([128, HID], F32, tag="op", space="PSUM")
                    nc.tensor.matmul(op_[:], lhsT=rT[:, 0:128], rhs=u2_sb[:, 0, :],
                                     start=True, stop=False)
                    nc.tensor.matmul(op_[:], lhsT=rT[:, 128:256], rhs=u2_sb[:, 1, :],
                                     start=False, stop=False)
                    nc.tensor.matmul(op_[:], lhsT=ones_bf[:], rhs=rowb(3),
                                     start=False, stop=True)

                    # LayerNorm stats via bn_stats (node-major: mean/var per node)
                    st6 = sp.tile([128, 6], F32, tag="st6")
                    nc.vector.bn_stats(out=st6[:], in_=op_[:])
                    mv = sp.tile([128, 2], F32, tag="mv")
                    nc.vector.bn_aggr(out=mv[:], in_=st6[:])
                    sd = sp.tile([128, 1], F32, tag="sd")
                    nc.scalar.activation(sd[:], mv[:, 1:2], AF.Sqrt,
                                         bias=eps_sb[:, :1])
                    rstd = sp.tile([128, 1], F32, tag="rstd")
                    nc.vector.reciprocal(rstd[:], sd[:])
                    nmr = sp.tile([128, 1], F32, tag="nmr")
                    nc.vector.tensor_scalar(out=nmr[:], in0=mv[:, 0:1],
                                            scalar1=rstd[:, :1], scalar2=-1.0,
                                            op0=OP.mult, op1=OP.mult)
                    zc = sp.tile([128, HID], BF16, tag="zc")
                    nc.scalar.activation(zc[:], op_[:], AF.Identity,
                                         scale=rstd[:, :1], bias=nmr[:, :1])
                    return zc

                def stage2(g, zc):
                    # transpose to feature-major; fused gamma/beta into z and
                    # beta_l-scaled output accumulation (both per-partition)
                    ztp = psp.tile([128, HID], BF16, tag="ztp", space="PSUM")
                    for f in range(2):
                        nc.tensor.transpose(
                            out=ztp[:, f * 128:(f + 1) * 128],
                            in_=zc[:, f * 128:(f + 1) * 128],
                            identity=ident_bf[:],
                        )
                    ab = sp.tile([128, HID], F32, tag="ab")
                    for f in range(2):
                        fs = slice(f * 128, (f + 1) * 128)
                        nc.scalar.activation(zT[:, fcols(f, g)], ztp[:, fs],
                                             AF.Identity,
                                             scale=cols_sb[:, 0 + f:1 + f],
                                             bias=cols_sb[:, 2 + f:3 + f])
                        nc.scalar.activation(ab[:, fs], ztp[:, fs], AF.Identity,
                                             scale=cols_sb[:, 4 + f:5 + f],
                                             bias=cols_sb[:, 6 + f:7 + f])
                    oav = oacc[:].rearrange("p (f n) -> p f n", f=2)[
                        :, :, g * 128:(g + 1) * 128]
                    nc.vector.tensor_tensor(
                        out=oav, in0=oav,
                        in1=ab[:].rearrange("p (f n) -> p f n", f=2), op=OP.add)

                pend = {}

                def finish_b(g, sT):
                    if "z" in pend:
                        stage2(*pend.pop("z"))
                    pend["z"] = (g, stage1(g, sT))

                do_pass(0, qfullA_cur, finish_a)
                holder = {}
                if wt_next is not None:
                    wtn = wt_next

                    def hook1(wtn=wtn, holder=holder):
                        holder["qa"] = emit_qa_groups(wtn)

                    def hook2(holder=holder):
                        holder["q"] = emit_qa_collective(*holder["qa"])
                    do_pass(1, qfullB, finish_b,
                            hooks=[(GA + 1, hook1), (GA + 9, hook2)])
                    stage2(*pend.pop("z"))
                    qfullA_cur = holder["q"]
                    wt_cur = wt_next
                else:
                    do_pass(1, qfullB, finish_b)
                    stage2(*pend.pop("z"))

            # flush output accumulator to DRAM (bf16 -> fp32)
            for g in range(G):
                for f in range(2):
                    fl = sp.tile([128, 128], F32, tag="fl")
                    nc.vector.tensor_copy(out=fl[:], in_=oacc[:, fcols(f, g)])
                    nc.sync.dma_start(out=out_d[:, fcols(f, g)], in_=fl[:])

    nc.compile()
    return nc


def _prep_inputs(inputs, B, seg_off, idx_pack, dstv_cols, cinv, totb, betas,
                 nlayers=L):
    x = np.asarray(inputs["x"], np.float32)
    Win = np.asarray(inputs["Win"], np.float32)
    bin_ = np.asarray(inputs["bin_"], np.float32)
    W1 = np.asarray(inputs["W1"], np.float32)
    b1 = np.asarray(inputs["b1"], np.float32)
    W2 = np.asarray(inputs["W2"], np.float32)
    b2 = np.asarray(inputs["b2"], np.float32)
    U1 = np.asarray(inputs["U1"], np.float32)
    c1 = np.asarray(inputs["c1"], np.float32)
    U2 = np.asarray(inputs["U2"], np.float32)
    c2 = np.asarray(inputs["c2"], np.float32)
    ln_g = np.asarray(inputs["ln_g"], np.float32)
    ln_b = np.asarray(inputs["ln_b"], np.float32)

    w1t = np.ascontiguousarray(
        W1[:nlayers].reshape(nlayers, 2, 128, 128).transpose(0, 2, 1, 3)).astype(NPBF)
    w2t = np.ascontiguousarray(W2[:nlayers].reshape(nlayers, 128, 2, 128)).astype(NPBF)
    u1t = np.ascontiguousarray(
        U1[:nlayers].reshape(nlayers, 2, 128, 2, 128).transpose(0, 2, 1, 3, 4)
        .reshape(nlayers, 128, 4, 128)).astype(NPBF)
    u2t = np.ascontiguousarray(
        U2[:nlayers].reshape(nlayers, 2, 128, HID).transpose(0, 2, 1, 3)).astype(NPBF)
    # per-partition LN columns [g0,g1,b0,b1,gB0,gB1,bB0,bB1] (B = *beta_{l+1})
    cols = np.zeros((nlayers, 128, 8), np.float32)
    gh = ln_g[:nlayers].reshape(nlayers, 2, 128)
    bh = ln_b[:nlayers].reshape(nlayers, 2, 128)
    bl = betas[1:nlayers + 1][:, None]
    for f in range(2):
        cols[:, :, 0 + f] = gh[:, f]
        cols[:, :, 2 + f] = bh[:, f]
        cols[:, :, 4 + f] = gh[:, f] * bl
        cols[:, :, 6 + f] = bh[:, f] * bl
    rowsb = np.zeros((nlayers, 4, HID), np.float32)
    rowsb[:, 0, :128] = b1[:nlayers]
    rowsb[:, 1] = b2[:nlayers]
    rowsb[:, 2] = c1[:nlayers]
    rowsb[:, 3] = c2[:nlayers]
    rowsb = rowsb.astype(NPBF)

    shared = {
        "win": Win, "binrow": bin_[None, :], "w1t": w1t, "w2t": w2t,
        "u1t": u1t, "u2t": u2t, "cols": cols,
        "rowsb": rowsb.reshape(nlayers, 4 * HID),
    }
    in_maps = []
    for r in range(CORES):
        xs = np.zeros((128, NPAD), np.float32)
        xs[:, :NPC] = x[r * NPC:(r + 1) * NPC].T
        m = dict(shared)
        m["xT"] = xs
        m["idxp"] = np.ascontiguousarray(idx_pack[r])
        m["dstv"] = np.ascontiguousarray(dstv_cols[r]).astype(NPBF)
        m["cinv"] = np.broadcast_to(cinv[r][None, :], (128, NPAD)).astype(NPBF)
        in_maps.append(m)
    return in_maps


def kernel(**inputs) -> np.ndarray:
    beta = np.asarray(inputs["beta"], np.float32)
    bmax = beta.max()
    e = np.exp(beta - bmax)
    betas = (e / e.sum()).astype(np.float32)

    import os
    nl = int(os.environ.get("KLAYERS", L))
    B, seg_off, idx_pack, dstv_cols, cinv, totb = _preprocess(inputs["edge_index"])
    nc = _build(B, seg_off, totb, betas, nlayers=nl)
    in_maps = _prep_inputs(inputs, B, seg_off, idx_pack, dstv_cols, cinv, totb,
                           betas, nlayers=nl)
    res = run_bass_kernel_spmd(nc, in_maps, core_ids=list(range(CORES)))
    globals()["LAST_EXEC_NS"] = res.exec_time_ns or res.mean_exec_time_ns
    globals()["LAST_RES"] = res
    # out is feature-major [128, 2*NPAD]: reshape to node-major [NPC, 256]
    out = np.concatenate([
        res.results[r]["out"].reshape(128, 2, NPAD).transpose(2, 1, 0)
        .reshape(NPAD, HID)[:NPC]
        for r in range(CORES)
    ], 0)
    return out.astype(np.float32)


if __name__ == "__main__":
    rng = np.random.default_rng(0)
    ins = {
        "x": rng.standard_normal((N, IN_CH), dtype=np.float32),
        "edge_index": rng.integers(0, N, size=(2, 800000)).astype(np.int32),
        "Win": rng.standard_normal((IN_CH, HID), dtype=np.float32) * 0.05,
        "bin_": np.zeros(HID, np.float32),
        "W1": rng.standard_normal((L, HID, MSG), dtype=np.float32) * 0.05,
        "b1": np.zeros((L, MSG), np.float32),
        "W2": rng.standard_normal((L, MSG, HID), dtype=np.float32) * 0.05,
        "b2": np.zeros((L, HID), np.float32),
        "U1": rng.standard_normal((L, HID, HID), dtype=np.float32) * 0.05,
        "c1": np.zeros((L, HID), np.float32),
        "U2": rng.standard_normal((L, HID, HID), dtype=np.float32) * 0.05,
        "c2": np.zeros((L, HID), np.float32),
        "ln_g": np.ones((L, HID), np.float32),
        "ln_b": np.zeros((L, HID), np.float32),
        "beta": 0.01 * rng.standard_normal(L + 1).astype(np.float32),
    }
    out = kernel(**ins)
    print(out.shape, out.dtype, np.abs(out).mean())
